# revision 17
# baseline (speedup 1.0000x reference)
"""Trainium2 Bass kernel for nn_Conv2dKan (KAN 3x3 conv, Chebyshev basis).

Math: out[b,o,l] = sum_{i,k} w[i,o,k]*(silu(p) + sum_n c[i,o,k,n]*T_n(tanh(p)))
where p are 3x3 unfold patches of x (pad=1). Since T_n(tanh(.)) are Chebyshev
polynomials of u = tanh(x), we convert the Chebyshev basis to monomials u^d
(d=0..8). The d=0 (constant) terms fold into a per-(o,tap) bias (monomial
features are all zero at zero-padding, and the constant feature is 1
everywhere, so the bias is position-independent). The whole op becomes a
3x3 conv with 144 input feature channels {silu(x_i), u_i^1..u_i^8} -> 32
outputs, executed as PSUM-accumulating matmuls over 9 shifted views of a
zero-padded feature tensor.

Sharding: data-parallel over batch, 2 batch elements per core across 8 cores.
"""

import numpy as np

import concourse.bacc as bacc
import concourse.bass as bass
import concourse.mybir as mybir
from concourse.tile import TileContext
from concourse.bass_utils import run_bass_kernel_spmd

N_CORES = 8
BL = 2            # batch per core
CIN = 16
COUT = 32
NB = 8            # number of Chebyshev basis functions
H = W = 64
HP, WP = H + 2, W + 2
LP = BL * HP * WP  # padded free size per (d,i) row
F32 = mybir.dt.float32
F32R = mybir.dt.float32r
AF = mybir.ActivationFunctionType


def _cheb_to_monomial():
    # A[n, d]: T_n(x) = sum_d A[n, d] x^d, n = 0..NB
    A = np.zeros((NB + 1, NB + 1), dtype=np.float64)
    A[0, 0] = 1.0
    A[1, 1] = 1.0
    for n in range(1, NB):
        A[n + 1, 1:] = 2 * A[n, :-1]
        A[n + 1] -= A[n - 1]
    return A


def _host_weights(w, c):
    """Build matmul weight blocks from (w, c).

    Returns:
      wa   [128, 288]: rows p = d*16 + i -> power (d+1) coeff w*gamma_{d+1};
                       cols = k*32 + o.
      wb   [16, 288]:  silu coefficients w[i,o,k], rows = i.
      bias [128, 1]:   p = j*32 + o -> sum_{i,k} w*gamma_0 (j = psum strip).
    """
    w_sq = np.asarray(w, np.float64)[..., 0]          # (i,o,k)
    c_sq = np.asarray(c, np.float64)[:, :, :, 0, :]   # (i,o,k,n), n = T_1..T_8
    A = _cheb_to_monomial()
    gamma = np.einsum('iokn,nd->iokd', c_sq, A[1:])   # (i,o,k,9), d=0..8
    wc = w_sq[..., None] * gamma                      # (i,o,k,d)

    wa = np.zeros((128, 9 * COUT), np.float32)
    for d in range(8):
        wa[d * 16:(d + 1) * 16] = (
            wc[..., d + 1].transpose(0, 2, 1).reshape(CIN, 9 * COUT))
    wb = np.ascontiguousarray(
        w_sq.transpose(0, 2, 1).reshape(CIN, 9 * COUT).astype(np.float32))
    bias_o = wc[..., 0].sum(axis=(0, 2))              # (o,)
    bias = np.ascontiguousarray(
        np.tile(bias_o, 4)[:, None].astype(np.float32))
    return wa, wb, bias


def _build_nc(sim_compat=False):
    nc = bacc.Bacc("TRN2", target_bir_lowering=False, debug=False)
    x = nc.dram_tensor("x", [BL, CIN, H, W], F32, kind="ExternalInput")
    wa = nc.dram_tensor("wa", [128, 9 * COUT], F32R, kind="ExternalInput")
    wb = nc.dram_tensor("wb", [CIN, 9 * COUT], F32R, kind="ExternalInput")
    bias = nc.dram_tensor("bias", [128, 1], F32, kind="ExternalInput")
    out = nc.dram_tensor("out", [BL, COUT, H, W], F32, kind="ExternalOutput")

    with TileContext(nc) as tc:
        with (
            tc.tile_pool(name="sing", bufs=1) as sing,
            tc.tile_pool(name="feat", bufs=1) as feat,
            tc.tile_pool(name="pp", bufs=4, space="PSUM") as pp,
            tc.tile_pool(name="outp", bufs=4) as outp,
        ):
            # --- weights / bias ---
            wa_s = sing.tile([128, 9 * COUT], F32R, name="wa_s")
            nc.sync.dma_start(out=wa_s[:, :], in_=wa[:, :])
            wb_s = sing.tile([CIN, 9 * COUT], F32R, name="wb_s")
            nc.sync.dma_start(out=wb_s[:, :], in_=wb[:, :])
            bias_s = sing.tile([128, 1], F32, name="bias_s")
            nc.sync.dma_start(out=bias_s[:, :], in_=bias[:, :])

            # --- load x as [(b i yb), (yy xx)] = [128, 1024] ---
            xt = sing.tile([128, 16 * W], F32, name="xt")
            x_r = x.rearrange("b i (yb yy) xx -> (b i yb) (yy xx)", yb=4)
            nc.sync.dma_start(out=xt[:, :], in_=x_r)
            xt_v = xt.rearrange("p (yy xx) -> p yy xx", yy=16)

            # --- elementwise features, all [128, 16*66] ---
            # Feature rows are 66 wide: 64 data cols + 2 zero cols. When the
            # scatter below streams them into the 66-stride padded layout,
            # those zeros land exactly on the pad border columns. All
            # features are f(0)=0, so the pad zeros of u/s0 propagate through
            # the power chain for free.
            FW = 16 * WP  # 1056
            u = sing.tile([128, FW], F32, name="u")
            s0 = sing.tile([128, FW], F32, name="s0")
            u2 = sing.tile([128, FW], F32, name="u2")
            u3 = sing.tile([128, FW], F32, name="u3")
            u4 = sing.tile([128, FW], F32, name="u4")
            u5 = sing.tile([128, FW], F32, name="u5")
            u6 = sing.tile([128, FW], F32, name="u6")
            u7 = sing.tile([128, FW], F32, name="u7")
            u8 = sing.tile([128, FW], F32, name="u8")
            u_v = u.rearrange("p (yy xx) -> p yy xx", yy=16)
            s0_v = s0.rearrange("p (yy xx) -> p yy xx", yy=16)
            nc.vector.memset(u_v[:, :, W:WP], 0.0)
            nc.vector.memset(s0_v[:, :, W:WP], 0.0)
            nc.scalar.activation(out=u_v[:, :, 0:W], in_=xt_v[:, :, :],
                                 func=AF.Tanh)
            if sim_compat:
                # CoreSim has no Silu LUT; silu(x) = x * sigmoid(x)
                nc.scalar.activation(out=s0_v[:, :, 0:W], in_=xt_v[:, :, :],
                                     func=AF.Sigmoid)
                nc.vector.tensor_mul(s0_v[:, :, 0:W], s0_v[:, :, 0:W],
                                     xt_v[:, :, :])
            else:
                nc.scalar.activation(out=s0_v[:, :, 0:W], in_=xt_v[:, :, :],
                                     func=AF.Silu)
            nc.scalar.activation(out=u2[:, :], in_=u[:, :], func=AF.Square)
            nc.vector.tensor_mul(u3[:, :], u[:, :], u2[:, :])
            nc.scalar.activation(out=u4[:, :], in_=u2[:, :], func=AF.Square)
            nc.vector.tensor_mul(u5[:, :], u[:, :], u4[:, :])
            nc.scalar.activation(out=u6[:, :], in_=u3[:, :], func=AF.Square)
            nc.vector.tensor_mul(u7[:, :], u3[:, :], u4[:, :])
            nc.scalar.activation(out=u8[:, :], in_=u4[:, :], func=AF.Square)

            # --- padded conv-layout feature tensors ---
            # FA partition p = d*16 + i, power d+1 (d = 0..7).
            # FB partitions 0..15: silu (scattered first, it is ready
            # earliest and shares SBUF write ports with FA's d=0/d=2 groups).
            FA = feat.tile([128, LP], F32R, name="FA")
            FB = feat.tile([CIN, LP], F32R, name="FB")
            FA_v = FA.rearrange("p (b yp xp) -> p b yp xp", b=BL, yp=HP)
            FB_v = FB.rearrange("p (b yp xp) -> p b yp xp", b=BL, yp=HP)

            # zero the one-element borders (features are 0 at zero-padding)
            for bb in range(BL):
                nc.gpsimd.memset(FA_v[:, bb, 0, :].bitcast(F32), 0.0)
                nc.gpsimd.memset(FA_v[:, bb, HP - 1, :].bitcast(F32), 0.0)
                nc.vector.memset(FA_v[:, bb, :, 0].bitcast(F32), 0.0)
                nc.vector.memset(FA_v[:, bb, :, WP - 1].bitcast(F32), 0.0)
                nc.gpsimd.memset(FB_v[:, bb, 0, :].bitcast(F32), 0.0)
                nc.gpsimd.memset(FB_v[:, bb, HP - 1, :].bitcast(F32), 0.0)
                nc.vector.memset(FB_v[:, bb, :, 0].bitcast(F32), 0.0)
                nc.vector.memset(FB_v[:, bb, :, WP - 1].bitcast(F32), 0.0)

            # --- scatter features into conv layout ---
            # src partition (i,yb) holds 16 padded rows (1056 contiguous);
            # dst partition i takes 4 yb-chunks back to back: one contiguous
            # 64*66-element run starting at padded (row 1, col 1). The
            # trailing zero pair of each 66-wide source row lands on the
            # (row y, col 65) and (row y+1, col 0) border cells.
            FA_r = FA.rearrange("p (b r) -> p b r", b=BL)
            FB_r = FB.rearrange("p (b r) -> p b r", b=BL)
            feats = [u, u2, u3, u4, u5, u6, u7, u8]
            for bb in range(BL):
                nc.sync.dma_start(
                    out=FB_r[:, bb, WP + 1:WP + 1 + H * WP],
                    in_=s0[bb * 64:(bb + 1) * 64, :].bitcast(F32R),
                )
            for d in range(8):
                for bb in range(BL):
                    nc.sync.dma_start(
                        out=FA_r[d * 16:(d + 1) * 16, bb, WP + 1:WP + 1 + H * WP],
                        in_=feats[d][bb * 64:(bb + 1) * 64, :].bitcast(F32R),
                    )

            # --- matmuls: 16 blocks of 8 output rows, 18 taps each ---
            # fp32r matmuls require PSUM dst partition 0, so each block uses
            # its own [32, 512] psum tile (one bank), no column packing.
            out_v = out.rearrange("b o (blk yy) xx -> b blk o (yy xx)", blk=8)
            for g in range(16):
                bb, blk = divmod(g, 8)
                y0 = blk * 8
                ps = pp.tile([32, 512], F32, name="ps", tag="ps")
                for t in range(9):
                    ky, kx = divmod(t, 3)
                    nc.tensor.matmul(
                        ps[:, :], lhsT=wa_s[:, t * 32:(t + 1) * 32],
                        rhs=FA_v[:, bb, y0 + ky:y0 + ky + 8, kx:kx + W],
                        start=(t == 0), stop=False, skip_group_check=True)
                for t in range(9):
                    ky, kx = divmod(t, 3)
                    nc.tensor.matmul(
                        ps[:, :], lhsT=wb_s[:, t * 32:(t + 1) * 32],
                        rhs=FB_v[:, bb, y0 + ky:y0 + ky + 8, kx:kx + W],
                        start=False, stop=(t == 8), skip_group_check=True)
                ot = outp.tile([32, 512], F32, name="ot", tag="ot")
                nc.vector.tensor_scalar_add(ot[:, :], ps[:, :], bias_s[0:32, :])
                nc.sync.dma_start(out=out_v[bb, blk, :, :], in_=ot[:, :])
    nc.compile()
    return nc


_NC_CACHE = None


def _run(x, w, c, **kw):
    global _NC_CACHE
    x = np.ascontiguousarray(np.asarray(x, np.float32))
    wa, wb, bias = _host_weights(np.asarray(w), np.asarray(c))
    if _NC_CACHE is None:
        _NC_CACHE = _build_nc()
    nc = _NC_CACHE
    in_maps = [
        {"x": np.ascontiguousarray(x[k * BL:(k + 1) * BL]),
         "wa": wa, "wb": wb, "bias": bias}
        for k in range(N_CORES)
    ]
    res = run_bass_kernel_spmd(nc, in_maps, core_ids=list(range(N_CORES)), **kw)
    return np.concatenate([r["out"] for r in res.results], axis=0), res


def kernel(x, w, c):
    return _run(x, w, c)[0]


# revision 18
# speedup vs baseline: 2.5075x; 2.5075x over previous
"""Trainium2 Bass kernel for nn_Conv2dKan (KAN 3x3 conv, Chebyshev basis).

Math: out[b,o,l] = sum_{i,k} w[i,o,k]*(silu(p) + sum_n c[i,o,k,n]*T_n(tanh(p)))
where p are 3x3 unfold patches of x (pad=1). Since T_n(tanh(.)) are Chebyshev
polynomials of u = tanh(x), we convert the Chebyshev basis to monomials u^d
(d=0..8). The d=0 (constant) terms fold into a per-o bias (monomial features
are all zero at zero-padding). The op becomes a 3x3 conv over feature
channels -> 32 outputs, executed as PSUM-accumulating bf16 matmuls over 9
shifted views of a zero-padded feature tensor.

Precision: matmuls run in bf16 with the dominant silu term split into
hi+lo bf16 features (and w into hi+lo weights), giving ~7e-4 rel err.
The tiny Chebyshev coefficients (|c| ~ 1e-3) tolerate plain bf16.

Sharding: data-parallel over batch, 2 batch elements per core across 8 cores.
"""

import numpy as np
import ml_dtypes

import concourse.bacc as bacc
import concourse.bass as bass
import concourse.mybir as mybir
from concourse.tile import TileContext
from concourse.bass_utils import run_bass_kernel_spmd

N_CORES = 8
BL = 2            # batch per core
CIN = 16
COUT = 32
NB = 8            # number of Chebyshev basis functions
H = W = 64
HP, WP = H + 2, W + 2
LP = BL * HP * WP  # padded free size per feature row
FW = 16 * WP       # elementwise tile free size (16 padded rows)
KB = 3 * CIN       # 48 rows in the B (silu hi/lo) chunk
F32 = mybir.dt.float32
BF16 = mybir.dt.bfloat16
AF = mybir.ActivationFunctionType
NPBF = ml_dtypes.bfloat16


def _cheb_to_monomial():
    # A[n, d]: T_n(x) = sum_d A[n, d] x^d, n = 0..NB
    A = np.zeros((NB + 1, NB + 1), dtype=np.float64)
    A[0, 0] = 1.0
    A[1, 1] = 1.0
    for n in range(1, NB):
        A[n + 1, 1:] = 2 * A[n, :-1]
        A[n + 1] -= A[n - 1]
    return A


def _host_weights(w, c):
    """Build matmul weight blocks from (w, c).

    Returns (bf16 unless noted):
      wa   [128, 288]: rows p = d*16 + i -> power (d+1) coeff w*gamma_{d+1};
                       cols = k*32 + o.
      wb   [48, 288]:  rows i: w_hi (for s_hi); rows 16+i: w_hi (for s_lo);
                       rows 32+i: w_lo (for s_hi again).
      bias [128, 1] f32: p = j*32 + o -> sum_{i,k} w*gamma_0 (j = psum strip).
    """
    w_sq = np.asarray(w, np.float64)[..., 0]          # (i,o,k)
    c_sq = np.asarray(c, np.float64)[:, :, :, 0, :]   # (i,o,k,n), n = T_1..T_8
    A = _cheb_to_monomial()
    gamma = np.einsum('iokn,nd->iokd', c_sq, A[1:])   # (i,o,k,9), d=0..8
    wc = w_sq[..., None] * gamma                      # (i,o,k,d)

    wa = np.zeros((128, 9 * COUT), NPBF)
    for d in range(8):
        wa[d * 16:(d + 1) * 16] = (
            wc[..., d + 1].transpose(0, 2, 1).reshape(CIN, 9 * COUT)
            .astype(NPBF))
    w_f32 = w_sq.astype(np.float32)
    w_hi = w_f32.astype(NPBF)
    w_lo = (w_f32 - w_hi.astype(np.float32)).astype(NPBF)
    wb = np.zeros((KB, 9 * COUT), NPBF)
    wb[0:16] = w_hi.transpose(0, 2, 1).reshape(CIN, 9 * COUT)
    wb[16:32] = w_hi.transpose(0, 2, 1).reshape(CIN, 9 * COUT)
    wb[32:48] = w_lo.transpose(0, 2, 1).reshape(CIN, 9 * COUT)
    bias_o = wc[..., 0].sum(axis=(0, 2))              # (o,)
    bias = np.ascontiguousarray(
        np.tile(bias_o, 4)[:, None].astype(np.float32))
    return wa, wb, bias


def _build_nc(sim_compat=False):
    nc = bacc.Bacc("TRN2", target_bir_lowering=False, debug=False)
    x = nc.dram_tensor("x", [BL, CIN, H, W], F32, kind="ExternalInput")
    wa = nc.dram_tensor("wa", [128, 9 * COUT], BF16, kind="ExternalInput")
    wb = nc.dram_tensor("wb", [KB, 9 * COUT], BF16, kind="ExternalInput")
    bias = nc.dram_tensor("bias", [128, 1], F32, kind="ExternalInput")
    out = nc.dram_tensor("out", [BL, COUT, H, W], F32, kind="ExternalOutput")

    with TileContext(nc) as tc:
        with (
            tc.tile_pool(name="sing", bufs=1) as sing,
            tc.tile_pool(name="feat", bufs=1) as feat,
            tc.tile_pool(name="pp", bufs=4, space="PSUM") as pp,
            tc.tile_pool(name="outp", bufs=4) as outp,
        ):
            # --- weights / bias ---
            wa_s = sing.tile([128, 9 * COUT], BF16, name="wa_s")
            nc.sync.dma_start(out=wa_s[:, :], in_=wa[:, :])
            wb_s = sing.tile([KB, 9 * COUT], BF16, name="wb_s")
            nc.sync.dma_start(out=wb_s[:, :], in_=wb[:, :])
            bias_s = sing.tile([128, 1], F32, name="bias_s")
            nc.sync.dma_start(out=bias_s[:, :], in_=bias[:, :])

            # --- load x as [(b i yb), (yy xx)] = [128, 1024] ---
            xt = sing.tile([128, 16 * W], F32, name="xt")
            x_r = x.rearrange("b i (yb yy) xx -> (b i yb) (yy xx)", yb=4)
            nc.sync.dma_start(out=xt[:, :], in_=x_r)
            xt_v = xt.rearrange("p (yy xx) -> p yy xx", yy=16)

            # --- elementwise features, [128, 16*66] ---
            # Feature rows are 66 wide: 64 data cols + 2 zero cols; the
            # scatter streams them so the zeros land on pad border columns.
            # All features are f(0)=0, so pad zeros propagate for free.
            u = sing.tile([128, FW], BF16, name="u")
            s0 = sing.tile([128, FW], F32, name="s0")
            s_hi = sing.tile([128, FW], BF16, name="s_hi")
            s_lo = sing.tile([128, FW], BF16, name="s_lo")
            u2 = sing.tile([128, FW], BF16, name="u2")
            u3 = sing.tile([128, FW], BF16, name="u3")
            u4 = sing.tile([128, FW], BF16, name="u4")
            u5 = sing.tile([128, FW], BF16, name="u5")
            u6 = sing.tile([128, FW], BF16, name="u6")
            u7 = sing.tile([128, FW], BF16, name="u7")
            u8 = sing.tile([128, FW], BF16, name="u8")
            u_v = u.rearrange("p (yy xx) -> p yy xx", yy=16)
            s0_v = s0.rearrange("p (yy xx) -> p yy xx", yy=16)
            nc.vector.memset(u_v[:, :, W:WP], 0.0)
            nc.vector.memset(s0_v[:, :, W:WP], 0.0)
            nc.scalar.activation(out=u_v[:, :, 0:W], in_=xt_v[:, :, :],
                                 func=AF.Tanh)
            if sim_compat:
                # CoreSim has no Silu LUT; silu(x) = x * sigmoid(x)
                nc.scalar.activation(out=s0_v[:, :, 0:W], in_=xt_v[:, :, :],
                                     func=AF.Sigmoid)
                nc.vector.tensor_mul(s0_v[:, :, 0:W], s0_v[:, :, 0:W],
                                     xt_v[:, :, :])
            else:
                nc.scalar.activation(out=s0_v[:, :, 0:W], in_=xt_v[:, :, :],
                                     func=AF.Silu)
            nc.vector.tensor_copy(s_hi[:, :], s0[:, :])
            nc.vector.tensor_sub(s_lo[:, :], s0[:, :], s_hi[:, :])
            nc.scalar.activation(out=u2[:, :], in_=u[:, :], func=AF.Square)
            nc.vector.tensor_mul(u3[:, :], u[:, :], u2[:, :])
            nc.scalar.activation(out=u4[:, :], in_=u2[:, :], func=AF.Square)
            nc.vector.tensor_mul(u5[:, :], u[:, :], u4[:, :])
            nc.scalar.activation(out=u6[:, :], in_=u3[:, :], func=AF.Square)
            nc.vector.tensor_mul(u7[:, :], u3[:, :], u4[:, :])
            nc.scalar.activation(out=u8[:, :], in_=u4[:, :], func=AF.Square)

            # --- padded conv-layout feature tensors (bf16) ---
            # FA partition p = d*16 + i (power d+1); FB rows: s_hi, s_lo, s_hi.
            FA = feat.tile([128, LP], BF16, name="FA")
            FB = feat.tile([KB, LP], BF16, name="FB")
            FA_v = FA.rearrange("p (b yp xp) -> p b yp xp", b=BL, yp=HP)
            FB_v = FB.rearrange("p (b yp xp) -> p b yp xp", b=BL, yp=HP)

            # zero the one-element borders (features are 0 at zero-padding)
            for bb in range(BL):
                nc.gpsimd.memset(FA_v[:, bb, 0, :], 0.0)
                nc.gpsimd.memset(FA_v[:, bb, HP - 1, :], 0.0)
                nc.vector.memset(FA_v[:, bb, :, 0], 0.0)
                nc.vector.memset(FA_v[:, bb, :, WP - 1], 0.0)
                nc.gpsimd.memset(FB_v[:, bb, 0, :], 0.0)
                nc.gpsimd.memset(FB_v[:, bb, HP - 1, :], 0.0)
                nc.vector.memset(FB_v[:, bb, :, 0], 0.0)
                nc.vector.memset(FB_v[:, bb, :, WP - 1], 0.0)

            # --- scatter features into conv layout ---
            # src partition (i,yb) holds 16 padded rows (1056 contiguous);
            # dst partition takes 4 yb-chunks back to back: one contiguous
            # 64*66 run starting at padded (row 1, col 1). Alternate the two
            # HWDGE rings (sync / scalar) to double descriptor throughput.
            FA_r = FA.rearrange("p (b r) -> p b r", b=BL)
            FB_r = FB.rearrange("p (b r) -> p b r", b=BL)
            feats = [u, u2, u3, u4, u5, u6, u7, u8]
            engs = [nc.sync, nc.scalar]
            n_dma = 0
            for bb in range(BL):
                for r0, ft in ((0, s_hi), (16, s_lo), (32, s_hi)):
                    engs[n_dma % 2].dma_start(
                        out=FB_r[r0:r0 + 16, bb, WP + 1:WP + 1 + H * WP],
                        in_=ft[bb * 64:(bb + 1) * 64, :])
                    n_dma += 1
                for d in range(8):
                    engs[n_dma % 2].dma_start(
                        out=FA_r[d * 16:(d + 1) * 16, bb,
                                 WP + 1:WP + 1 + H * WP],
                        in_=feats[d][bb * 64:(bb + 1) * 64, :])
                    n_dma += 1

            # --- matmuls: 4 groups x (4 col strips x 18 taps) ---
            # Each group: one [128, 512] psum bank; strip j = output rows
            # [y0+8j, y0+8j+8) of batch bb. Tap-major issue order puts
            # consecutive matmuls in different column groups so they stream
            # concurrently through the PE array.
            out_v = out.rearrange("b o (q j yy) xx -> b q j o (yy xx)",
                                  q=2, j=4)
            for g in range(4):
                bb, q = divmod(g, 2)
                ps = pp.tile([128, 512], F32, name="ps", tag="ps")
                for t in range(9):
                    ky, kx = divmod(t, 3)
                    lhsA = wa_s[:, t * 32:(t + 1) * 32]
                    for j in range(4):
                        y0 = q * 32 + j * 8
                        nc.tensor.matmul(
                            ps[j * 32:(j + 1) * 32, :], lhsT=lhsA,
                            rhs=FA_v[:, bb, y0 + ky:y0 + ky + 8, kx:kx + W],
                            start=(t == 0), stop=False, skip_group_check=True,
                            tile_position=(0, 32 * j))
                for t in range(9):
                    ky, kx = divmod(t, 3)
                    lhsB = wb_s[:, t * 32:(t + 1) * 32]
                    for j in range(4):
                        y0 = q * 32 + j * 8
                        nc.tensor.matmul(
                            ps[j * 32:(j + 1) * 32, :], lhsT=lhsB,
                            rhs=FB_v[:, bb, y0 + ky:y0 + ky + 8, kx:kx + W],
                            start=False, stop=(t == 8), skip_group_check=True,
                            tile_position=(0, 32 * j))
                ot = outp.tile([128, 512], F32, name="ot", tag="ot")
                nc.vector.tensor_scalar_add(ot[:, :], ps[:, :], bias_s[:, :])
                engs[g % 2].dma_start(out=out_v[bb, q, :, :, :], in_=ot[:, :])
    nc.compile()
    return nc


_NC_CACHE = None


def _run(x, w, c, **kw):
    global _NC_CACHE
    x = np.ascontiguousarray(np.asarray(x, np.float32))
    wa, wb, bias = _host_weights(np.asarray(w), np.asarray(c))
    if _NC_CACHE is None:
        _NC_CACHE = _build_nc()
    nc = _NC_CACHE
    in_maps = [
        {"x": np.ascontiguousarray(x[k * BL:(k + 1) * BL]),
         "wa": wa, "wb": wb, "bias": bias}
        for k in range(N_CORES)
    ]
    res = run_bass_kernel_spmd(nc, in_maps, core_ids=list(range(N_CORES)), **kw)
    return np.concatenate([r["out"] for r in res.results], axis=0), res


def kernel(x, w, c):
    return _run(x, w, c)[0]


# revision 23
# speedup vs baseline: 2.5990x; 1.0365x over previous
"""Trainium2 Bass kernel for nn_Conv2dKan (KAN 3x3 conv, Chebyshev basis).

Math: out[b,o,l] = sum_{i,k} w[i,o,k]*(silu(p) + sum_n c[i,o,k,n]*T_n(tanh(p)))
where p are 3x3 unfold patches of x (pad=1). Since T_n(tanh(.)) are Chebyshev
polynomials of u = tanh(x), we convert the Chebyshev basis to monomials u^d
(d=0..8). The d=0 (constant) terms fold into a per-o bias (monomial features
are all zero at zero-padding). The op becomes a 3x3 conv over feature
channels -> 32 outputs, executed as PSUM-accumulating bf16 matmuls over 9
shifted views of a zero-padded feature tensor.

Precision: matmuls run in bf16 with the dominant silu term split into
hi+lo bf16 features (and w into hi+lo weights), giving ~7e-4 rel err.
The tiny Chebyshev coefficients (|c| ~ 1e-3) tolerate plain bf16.

Sharding: data-parallel over batch, 2 batch elements per core across 8 cores.
"""

import numpy as np
import ml_dtypes

import concourse.bacc as bacc
import concourse.bass as bass
import concourse.mybir as mybir
from concourse.tile import TileContext
from concourse.bass_utils import run_bass_kernel_spmd

N_CORES = 8
BL = 2            # batch per core
CIN = 16
COUT = 32
NB = 8            # number of Chebyshev basis functions
H = W = 64
HP, WP = H + 2, W + 2
LP = BL * HP * WP  # padded free size per feature row
FW = 16 * WP       # elementwise tile free size (16 padded rows)
KB = 3 * CIN       # 48 rows in the B (silu hi/lo) chunk
F32 = mybir.dt.float32
BF16 = mybir.dt.bfloat16
AF = mybir.ActivationFunctionType
NPBF = ml_dtypes.bfloat16


def _cheb_to_monomial():
    # A[n, d]: T_n(x) = sum_d A[n, d] x^d, n = 0..NB
    A = np.zeros((NB + 1, NB + 1), dtype=np.float64)
    A[0, 0] = 1.0
    A[1, 1] = 1.0
    for n in range(1, NB):
        A[n + 1, 1:] = 2 * A[n, :-1]
        A[n + 1] -= A[n - 1]
    return A


def _host_weights(w, c):
    """Build matmul weight blocks from (w, c).

    Returns (bf16 unless noted):
      wa   [128, 288]: rows p = d*16 + i -> power (d+1) coeff w*gamma_{d+1};
                       cols = k*32 + o.
      wb   [48, 288]:  rows i: w_hi (for s_hi); rows 16+i: w_hi (for s_lo);
                       rows 32+i: w_lo (for s_hi again).
      bias [128, 1] f32: p = j*32 + o -> sum_{i,k} w*gamma_0 (j = psum strip).
    """
    w_sq = np.asarray(w, np.float64)[..., 0]          # (i,o,k)
    c_sq = np.asarray(c, np.float64)[:, :, :, 0, :]   # (i,o,k,n), n = T_1..T_8
    A = _cheb_to_monomial()
    gamma = np.einsum('iokn,nd->iokd', c_sq, A[1:])   # (i,o,k,9), d=0..8
    wc = w_sq[..., None] * gamma                      # (i,o,k,d)

    wa = np.zeros((128, 9 * COUT), NPBF)
    for d in range(8):
        wa[d * 16:(d + 1) * 16] = (
            wc[..., d + 1].transpose(0, 2, 1).reshape(CIN, 9 * COUT)
            .astype(NPBF))
    w_f32 = w_sq.astype(np.float32)
    w_hi = w_f32.astype(NPBF)
    w_lo = (w_f32 - w_hi.astype(np.float32)).astype(NPBF)
    wb = np.zeros((KB, 9 * COUT), NPBF)
    wb[0:16] = w_hi.transpose(0, 2, 1).reshape(CIN, 9 * COUT)
    wb[16:32] = w_hi.transpose(0, 2, 1).reshape(CIN, 9 * COUT)
    wb[32:48] = w_lo.transpose(0, 2, 1).reshape(CIN, 9 * COUT)
    bias_o = wc[..., 0].sum(axis=(0, 2))              # (o,)
    bias = np.ascontiguousarray(
        np.tile(bias_o, 4)[:, None].astype(np.float32))
    return wa, wb, bias


def _build_nc(sim_compat=False):
    nc = bacc.Bacc("TRN2", target_bir_lowering=False, debug=False)
    x = nc.dram_tensor("x", [BL, CIN, H, W], F32, kind="ExternalInput")
    wa = nc.dram_tensor("wa", [128, 9 * COUT], BF16, kind="ExternalInput")
    wb = nc.dram_tensor("wb", [KB, 9 * COUT], BF16, kind="ExternalInput")
    bias = nc.dram_tensor("bias", [128, 1], F32, kind="ExternalInput")
    out = nc.dram_tensor("out", [BL, COUT, H, W], F32, kind="ExternalOutput")

    with TileContext(nc) as tc:
        with (
            tc.tile_pool(name="sing", bufs=1) as sing,
            tc.tile_pool(name="feat", bufs=1) as feat,
            tc.tile_pool(name="pp", bufs=4, space="PSUM") as pp,
            tc.tile_pool(name="outp", bufs=4) as outp,
        ):
            # --- weights / bias ---
            wa_s = sing.tile([128, 9 * COUT], BF16, name="wa_s")
            nc.sync.dma_start(out=wa_s[:, :], in_=wa[:, :])
            wb_s = sing.tile([KB, 9 * COUT], BF16, name="wb_s")
            nc.sync.dma_start(out=wb_s[:, :], in_=wb[:, :])
            bias_s = sing.tile([128, 1], F32, name="bias_s")
            nc.sync.dma_start(out=bias_s[:, :], in_=bias[:, :])

            # --- load x as [(b i yb), (yy xx)] = [128, 1024] ---
            xt = sing.tile([128, 16 * W], F32, name="xt")
            x_r = x.rearrange("b i (yb yy) xx -> (b i yb) (yy xx)", yb=4)
            nc.sync.dma_start(out=xt[:, :], in_=x_r)
            xt_v = xt.rearrange("p (yy xx) -> p yy xx", yy=16)

            # --- elementwise features, [128, 16*66] ---
            # Feature rows are 66 wide: 64 data cols + 2 zero cols; the
            # scatter streams them so the zeros land on pad border columns.
            # All features are f(0)=0, so pad zeros propagate for free.
            u = sing.tile([128, FW], BF16, name="u")
            s0 = sing.tile([128, FW], F32, name="s0")
            s_hi = sing.tile([128, FW], BF16, name="s_hi")
            s_lo = sing.tile([128, FW], BF16, name="s_lo")
            u2 = sing.tile([128, FW], BF16, name="u2")
            u3 = sing.tile([128, FW], BF16, name="u3")
            u4 = sing.tile([128, FW], BF16, name="u4")
            u5 = sing.tile([128, FW], BF16, name="u5")
            u6 = sing.tile([128, FW], BF16, name="u6")
            u7 = sing.tile([128, FW], BF16, name="u7")
            u8 = sing.tile([128, FW], BF16, name="u8")
            u_v = u.rearrange("p (yy xx) -> p yy xx", yy=16)
            s0_v = s0.rearrange("p (yy xx) -> p yy xx", yy=16)
            nc.vector.memset(u_v[:, :, W:WP], 0.0)
            nc.vector.memset(s0_v[:, :, W:WP], 0.0)
            # Power chain ordered so u8 (the critical path to the first
            # matmul) completes in 4 back-to-back ACT ops; u6 computes on
            # DVE (u2*u4) to keep it off that path. silu (only needed by
            # the B-chunk matmuls, which run last in each group) follows.
            nc.scalar.activation(out=u_v[:, :, 0:W], in_=xt_v[:, :, :],
                                 func=AF.Tanh)
            nc.scalar.activation(out=u2[:, :], in_=u[:, :], func=AF.Square)
            nc.scalar.activation(out=u4[:, :], in_=u2[:, :], func=AF.Square)
            nc.scalar.activation(out=u8[:, :], in_=u4[:, :], func=AF.Square)
            if sim_compat:
                # CoreSim has no Silu LUT; silu(x) = x * sigmoid(x)
                nc.scalar.activation(out=s0_v[:, :, 0:W], in_=xt_v[:, :, :],
                                     func=AF.Sigmoid)
                nc.vector.tensor_mul(s0_v[:, :, 0:W], s0_v[:, :, 0:W],
                                     xt_v[:, :, :])
            else:
                nc.scalar.activation(out=s0_v[:, :, 0:W], in_=xt_v[:, :, :],
                                     func=AF.Silu)
            nc.vector.tensor_mul(u3[:, :], u[:, :], u2[:, :])
            nc.vector.tensor_mul(u5[:, :], u[:, :], u4[:, :])
            nc.vector.tensor_mul(u6[:, :], u2[:, :], u4[:, :])
            nc.vector.tensor_mul(u7[:, :], u3[:, :], u4[:, :])
            nc.vector.tensor_copy(s_hi[:, :], s0[:, :])
            nc.vector.tensor_sub(s_lo[:, :], s0[:, :], s_hi[:, :])

            # --- padded conv-layout feature tensors (bf16) ---
            # FA partition p = d*16 + i (power d+1); FB rows: s_hi, s_lo, s_hi.
            FA = feat.tile([128, LP], BF16, name="FA")
            FB = feat.tile([KB, LP], BF16, name="FB")
            FA_v = FA.rearrange("p (b yp xp) -> p b yp xp", b=BL, yp=HP)
            FB_v = FB.rearrange("p (b yp xp) -> p b yp xp", b=BL, yp=HP)

            # zero the one-element borders (features are 0 at zero-padding)
            for bb in range(BL):
                nc.gpsimd.memset(FA_v[:, bb, 0, :], 0.0)
                nc.gpsimd.memset(FA_v[:, bb, HP - 1, :], 0.0)
                nc.vector.memset(FA_v[:, bb, :, 0], 0.0)
                nc.vector.memset(FA_v[:, bb, :, WP - 1], 0.0)
                nc.gpsimd.memset(FB_v[:, bb, 0, :], 0.0)
                nc.gpsimd.memset(FB_v[:, bb, HP - 1, :], 0.0)
                nc.vector.memset(FB_v[:, bb, :, 0], 0.0)
                nc.vector.memset(FB_v[:, bb, :, WP - 1], 0.0)

            # --- scatter features into conv layout ---
            # src partition (i,yb) holds 16 padded rows (1056 contiguous);
            # dst partition takes 4 yb-chunks back to back: one contiguous
            # 64*66 run starting at padded (row 1, col 1). Alternate the two
            # HWDGE rings (sync / scalar) to double descriptor throughput.
            FA_r = FA.rearrange("p (b r) -> p b r", b=BL)
            FB_r = FB.rearrange("p (b r) -> p b r", b=BL)
            feats = [u, u2, u3, u4, u5, u6, u7, u8]
            engs = [nc.sync, nc.scalar]
            # Issue order = producer readiness order (HWDGE queues are FIFO
            # per engine; a not-yet-ready DMA at the queue head blocks the
            # rest). b=0 first so its matmul groups can start early.
            sched = [(0, feats[0]), (1, feats[1]), (2, feats[2]),
                     (3, feats[3]), (7, feats[7]), (4, feats[4]),
                     (5, feats[5]), (6, feats[6]),
                     (8, s_hi), (9, s_lo), (10, s_hi)]
            n_dma = 0
            for bb in range(BL):
                for slot, ft in sched:
                    dst = (FA_r[slot * 16:(slot + 1) * 16, bb,
                                WP + 1:WP + 1 + H * WP]
                           if slot < 8 else
                           FB_r[(slot - 8) * 16:(slot - 7) * 16, bb,
                                WP + 1:WP + 1 + H * WP])
                    engs[n_dma % 2].dma_start(
                        out=dst, in_=ft[bb * 64:(bb + 1) * 64, :])
                    n_dma += 1

            # --- PE warmup: dummy matmuls keep the HAM activity monitor
            # busy while the scatter runs, so real matmuls start at the
            # warm 2.4 GHz clock instead of spending ~12 us cold. They
            # read the (already loaded) weight tile and write a scratch
            # psum bank nothing reads.
            ps_warm = pp.tile([32, 512], F32, name="ps_warm", tag="warm",
                              bufs=1)
            for _ in range(60):
                nc.tensor.matmul(
                    ps_warm[:, 0:288], lhsT=wa_s[:, 0:32], rhs=wa_s[:, :],
                    start=True, stop=True, skip_group_check=True)

            # --- matmuls: 4 groups x (4 col strips x 18 taps) ---
            # Each group: one [128, 512] psum bank; strip j = output rows
            # [y0+8j, y0+8j+8) of batch bb. Tap-major issue order puts
            # consecutive matmuls in different column groups so they stream
            # concurrently through the PE array.
            out_v = out.rearrange("b o (q j yy) xx -> b q j o (yy xx)",
                                  q=2, j=4)
            for g in range(4):
                bb, q = divmod(g, 2)
                ps = pp.tile([128, 512], F32, name="ps", tag="ps")
                for t in range(9):
                    ky, kx = divmod(t, 3)
                    lhsA = wa_s[:, t * 32:(t + 1) * 32]
                    for j in range(4):
                        y0 = q * 32 + j * 8
                        nc.tensor.matmul(
                            ps[j * 32:(j + 1) * 32, :], lhsT=lhsA,
                            rhs=FA_v[:, bb, y0 + ky:y0 + ky + 8, kx:kx + W],
                            start=(t == 0), stop=False, skip_group_check=True,
                            tile_position=(0, 32 * j))
                for t in range(9):
                    ky, kx = divmod(t, 3)
                    lhsB = wb_s[:, t * 32:(t + 1) * 32]
                    for j in range(4):
                        y0 = q * 32 + j * 8
                        nc.tensor.matmul(
                            ps[j * 32:(j + 1) * 32, :], lhsT=lhsB,
                            rhs=FB_v[:, bb, y0 + ky:y0 + ky + 8, kx:kx + W],
                            start=False, stop=(t == 8), skip_group_check=True,
                            tile_position=(0, 32 * j))
                ot = outp.tile([128, 512], F32, name="ot", tag="ot")
                nc.vector.tensor_scalar_add(ot[:, :], ps[:, :], bias_s[:, :])
                engs[g % 2].dma_start(out=out_v[bb, q, :, :, :], in_=ot[:, :])
    nc.compile()
    return nc


_NC_CACHE = None


def _run(x, w, c, **kw):
    global _NC_CACHE
    x = np.ascontiguousarray(np.asarray(x, np.float32))
    wa, wb, bias = _host_weights(np.asarray(w), np.asarray(c))
    if _NC_CACHE is None:
        _NC_CACHE = _build_nc()
    nc = _NC_CACHE
    in_maps = [
        {"x": np.ascontiguousarray(x[k * BL:(k + 1) * BL]),
         "wa": wa, "wb": wb, "bias": bias}
        for k in range(N_CORES)
    ]
    res = run_bass_kernel_spmd(nc, in_maps, core_ids=list(range(N_CORES)), **kw)
    return np.concatenate([r["out"] for r in res.results], axis=0), res


def kernel(x, w, c):
    return _run(x, w, c)[0]
